# revision 15
# baseline (speedup 1.0000x reference)
"""Trainium2 Bass kernel for the attention module:

    att_h  = h @ W_h2att.T + b_h2att             # [B, 512]
    dot    = tanh(p_att_feats + att_h[:, None])  # [B, 1024, 512]
    scores = dot @ w_alpha + b_alpha             # [B, 1024]
    weight = softmax(scores, axis=1)
    out    = einsum('bs,bsd->bd', weight, att_feats)  # [B, 2048]

Sharding: data-parallel over batch B=64 across 8 NeuronCores (8 per core).
Params tiny + replicated. b_alpha is a softmax shift -> dropped.

v10 design (~32MB/core HBM read; DMA floor ~95us):
  - hybrid att precision: s-chunks c=0,1 stay bf16 (consumed directly);
    c=2..7 host-quantized int8 + per-(b,s)-row absmax/127 scale.
    Upconverts: ACT does c2,c3 (early-landing), DVE does c4..c7.
    Pool only issues pT DMAs (its software tensor_copy is ~3x the
    cost-model estimate and poisons the tile schedule).
  - every DMA has >=2KB contiguous per-partition lines (WT/hT/pT are
    host-arranged partition-major; fragmented APs run ~4x slower).
  - int8 pools split by reader engine (a8A/a8D) so one engine's lag
    can't stall the other's ring recycle.
  - int8 chunks DMA first / bf16 last per batch; the weighted matmuls
    consume in the same order, hiding convert latency.
  - weighted sum accumulates all batches into persistent [8, 512] PSUM
    bank tiles via zero-masked weight columns (lhsT [128, 8], only
    column b nonzero); single tail copy + one output DMA.
  - ACT computes tanh(pT + att_h[b,h]) with att_h as per-partition bias;
    scores via PE matvec (s = c*128 + q); exp 2 iterations ahead of use.
  - DMA queues: att on SP ring, wt on ACT ring, pT/consts on gpsimd.
"""

import numpy as np
import ml_dtypes

import concourse.bass as bass
import concourse.tile as tile
from concourse import bacc, mybir
from concourse.bass import ts
from concourse.bass_utils import run_bass_kernel_spmd

F32 = mybir.dt.float32
BF16 = mybir.dt.bfloat16
I8 = mybir.dt.int8

B_LOC = 8       # batches per core
S = 1024        # attended positions
NC_ = 8         # s-chunks (s = c*128 + q)
NBF = 2         # bf16-direct s-chunks per batch (c = 0..NBF-1)
NQ = NC_ - NBF  # int8 s-chunks per batch (c = NBF..7)
HID = 512
NHC = 4         # h-chunks
D = 2048
DT = D // 512   # output column slices
K = 2048        # rnn_size contraction
KG = K // 128   # 16 k-groups

_NC_CACHE = None


def build_kernel(abf_bufs=3, a8a_bufs=4, a8d_bufs=7, aba_bufs=3, abd_bufs=10,
                 pt_bufs=3, th_bufs=3):
    nc = bacc.Bacc("TRN2", target_bir_lowering=False, debug=False, num_devices=8)

    # pT2 host layout: [b, q, hc, s] (partition-major, 8KB lines)
    p_d = nc.dram_tensor("pT2", [B_LOC, 128, NHC, S], BF16, kind="ExternalInput")
    # packed layouts: per-partition lines are contiguous (8KB / 4KB)
    abf_d = nc.dram_tensor("attbf", [B_LOC, 128, NBF, D], BF16, kind="ExternalInput")
    a8_d = nc.dram_tensor("att8", [B_LOC, 128, NQ // 2, 2, D], I8, kind="ExternalInput")
    scl_d = nc.dram_tensor("scl", [128, B_LOC, NQ], F32, kind="ExternalInput")
    # hT2 host layout: [q, kg, b]; WT2 host layout: [q, kg, h]
    hT_d = nc.dram_tensor("hT2", [128, KG, B_LOC], BF16, kind="ExternalInput")
    WT_d = nc.dram_tensor("WT2", [128, KG, HID], BF16, kind="ExternalInput")
    wa_d = nc.dram_tensor("wa", [128, NHC], BF16, kind="ExternalInput")
    bh_d = nc.dram_tensor("bh", [128, NHC], F32, kind="ExternalInput")
    out_d = nc.dram_tensor("out", [B_LOC, D], F32, kind="ExternalOutput")
    z_d = nc.dram_tensor("zall", [128, B_LOC], F32, kind="ExternalOutput")

    with tile.TileContext(nc) as tc:
        with (
            tc.tile_pool(name="consts", bufs=1) as consts,
            tc.tile_pool(name="singles", bufs=1) as singles,
            tc.tile_pool(name="ptp", bufs=pt_bufs) as ptpool,
            tc.tile_pool(name="thp", bufs=th_bufs) as thpool,
            tc.tile_pool(name="wgtp", bufs=3) as wgtpool,
            tc.tile_pool(name="wgmp", bufs=3) as wgmpool,
            tc.tile_pool(name="abfp", bufs=abf_bufs) as abfpool,
            tc.tile_pool(name="a8ap", bufs=a8a_bufs) as a8apool,
            tc.tile_pool(name="a8dp", bufs=a8d_bufs) as a8dpool,
            tc.tile_pool(name="abap", bufs=aba_bufs) as abapool,
            tc.tile_pool(name="abdp", bufs=abd_bufs) as abdpool,
            tc.tile_pool(name="ps_ah", bufs=1, space=bass.MemorySpace.PSUM) as ps_ah,
            tc.tile_pool(name="ps_sc", bufs=2, space=bass.MemorySpace.PSUM) as ps_sc,
            tc.tile_pool(name="ps_acc", bufs=1, space=bass.MemorySpace.PSUM) as ps_acc,
        ):
            abf_tiles = {}
            a8_tiles = {}

            def emit_att_dma(b):
                # int8 pairs on the SP ring (4KB lines); the bf16 pair c0-1
                # goes on the ACT ring (8KB lines) which also carried wt
                ta = a8apool.tile([128, 2, D], I8, name=f"a8a{b}", tag="a8a")
                nc.sync.dma_start(ta[:], a8_d[b][:, 0, :, :])
                t1 = a8dpool.tile([128, 2, D], I8, name=f"a8d{b}_0", tag="a8d")
                nc.sync.dma_start(t1[:], a8_d[b][:, 1, :, :])
                t2 = a8dpool.tile([128, 2, D], I8, name=f"a8d{b}_1", tag="a8d")
                nc.sync.dma_start(t2[:], a8_d[b][:, 2, :, :])
                a8_tiles[b] = [ta, t1, t2]
                at = abfpool.tile([128, NBF, D], BF16, name=f"abf{b}", tag="abf")
                nc.scalar.dma_start(at[:], abf_d[b])
                abf_tiles[b] = at

            ab_tiles = {}

            def emit_convert(b, engines="AD"):
                tiles = ab_tiles.setdefault(b, {})
                for c in range(NBF, NC_):
                    i = c - NBF
                    eng = "A" if c == 2 else "D"
                    if eng not in engines:
                        continue
                    src = a8_tiles[b][i // 2][:, i % 2, :]
                    if eng == "A":
                        dst = abapool.tile([128, D], BF16, name=f"aba{b}_{c}", tag="aba")
                        nc.scalar.copy(dst[:], src)
                    else:
                        dst = abdpool.tile([128, D], BF16, name=f"abd{b}_{c}", tag="abd")
                        nc.vector.tensor_copy(dst[:], src)
                    tiles[c] = dst

            pt_tiles = {}

            def emit_pt_dma(b):
                pt = ptpool.tile([128, NHC, S], BF16, name=f"pt{b}", tag="pt")
                nc.gpsimd.dma_start(pt[:], p_d[b])
                pt_tiles[b] = pt

            # ---- consts: tiny ones on gpsimd ring, wt on ACT ring;
            # the att stream starts immediately on SP
            ht = consts.tile([128, KG, B_LOC], BF16)
            nc.gpsimd.dma_start(ht[:], hT_d[:])
            wa = consts.tile([128, NHC], BF16)
            nc.gpsimd.dma_start(wa[:], wa_d[:])
            bh = consts.tile([128, NHC], F32)
            nc.gpsimd.dma_start(bh[:], bh_d[:])
            scl = consts.tile([128, B_LOC, NQ], F32)
            nc.gpsimd.dma_start(scl[:], scl_d[:])

            wt = consts.tile([128, KG, HID], BF16)
            nc.scalar.dma_start(wt[:], WT_d[:])

            emit_att_dma(0)
            emit_pt_dma(0)
            emit_att_dma(1)
            emit_pt_dma(1)

            # ---- att_hT[h, b] = sum_k WT[k, h] * hT[k, b]  ([128, 4hc, 8b])
            ahT = ps_ah.tile([128, NHC * B_LOC], F32)
            for hc in range(NHC):
                for kg in range(KG):
                    nc.tensor.matmul(
                        ahT[:, ts(hc, B_LOC)],
                        wt[:, kg, ts(hc, 128)],
                        ht[:, kg, :],
                        start=(kg == 0),
                        stop=(kg == KG - 1),
                    )
            biasT = singles.tile([128, NHC, B_LOC], F32)
            for hc in range(NHC):
                nc.vector.tensor_add(
                    biasT[:, hc, :],
                    ahT[:, ts(hc, B_LOC)],
                    bh[:, hc : hc + 1].broadcast_to((128, B_LOC)),
                )

            # per-partition exp partial sums, one column per batch
            zall = singles.tile([128, B_LOC], F32)

            # persistent weighted-sum accumulator: [8 batches, 4 dchunks, 512]
            acc = ps_acc.tile([B_LOC, DT, 512], F32)

            th_tiles = {}
            sc_tiles = {}
            wgt_tiles = {}
            wgm_tiles = {}

            def emit_tanh(b):
                th = thpool.tile([128, NHC, S], BF16, name=f"th{b}", tag="th")
                for hc in range(NHC):
                    nc.scalar.activation(
                        th[:, hc, :], pt_tiles[b][:, hc, :],
                        mybir.ActivationFunctionType.Tanh,
                        bias=biasT[:, hc, b : b + 1], scale=1.0,
                    )
                th_tiles[b] = th

            def emit_scores(b):
                sc = ps_sc.tile([128, NC_], F32, name=f"sc{b}", tag="sc")
                for c in range(NC_):
                    for hc in range(NHC):
                        nc.tensor.matmul(
                            sc[:, c : c + 1],
                            th_tiles[b][:, hc, ts(c, 128)],
                            wa[:, hc : hc + 1],
                            start=(hc == 0),
                            stop=(hc == NHC - 1),
                        )
                sc_tiles[b] = sc

            def emit_exp(b):
                wgt = wgtpool.tile([128, NC_], BF16, name=f"wgt{b}", tag="wgt")
                nc.scalar.activation(
                    wgt[:], sc_tiles[b][:], mybir.ActivationFunctionType.Exp,
                    accum_out=zall[:, b : b + 1],
                )
                wgt_tiles[b] = wgt

            def emit_wgm(b):
                # masked lhsT: [128, c, 8] zero except column b; int8 chunks
                # get the dequant row scale folded in here
                wgm = wgmpool.tile(
                    [128, NC_, B_LOC], BF16, name=f"wgm{b}", tag="wgm"
                )
                nc.vector.memset(wgm[:], 0.0)
                nc.vector.tensor_copy(
                    wgm[:, 0:NBF, b], wgt_tiles[b][:, 0:NBF]
                )
                nc.vector.tensor_mul(
                    wgm[:, NBF:NC_, b], wgt_tiles[b][:, NBF:NC_], scl[:, b, :]
                )
                wgm_tiles[b] = wgm

            def emit_weighted(b):
                for c in [2, 3, 4, 5, 6, 7, 0, 1]:
                    if c < NBF:
                        at = abf_tiles[b][:, c, :]
                    else:
                        at = ab_tiles[b][c][:]
                    for d in range(DT):
                        nc.tensor.matmul(
                            acc[:, d, :],
                            wgm_tiles[b][:, c, :],
                            at[:, ts(d, 512)],
                            start=(b == 0 and c == 2),
                            stop=(b == B_LOC - 1 and c == 1),
                            skip_group_check=True,
                        )

            # ---- prologue
            emit_convert(0, "D")
            emit_tanh(0)
            emit_scores(0)
            emit_exp(0)
            emit_wgm(0)
            emit_convert(0, "A")
            emit_pt_dma(2)
            emit_tanh(1)
            emit_scores(1)
            emit_exp(1)
            emit_wgm(1)
            emit_convert(1, "D")

            for b in range(B_LOC):
                if b + 2 < B_LOC:
                    emit_att_dma(b + 2)
                emit_weighted(b)
                if b + 2 < B_LOC:
                    emit_tanh(b + 2)
                if b + 3 < B_LOC:
                    emit_pt_dma(b + 3)
                if b + 1 < B_LOC:
                    emit_convert(b + 1, "AD" if b > 0 else "A")
                if b + 2 < B_LOC:
                    emit_scores(b + 2)
                    emit_exp(b + 2)
                    emit_wgm(b + 2)

            # tail: drain the persistent accumulator
            rowall = singles.tile([B_LOC, D], F32)
            for d in range(DT):
                nc.vector.tensor_copy(rowall[:, ts(d, 512)], acc[:, d, :])
            nc.sync.dma_start(out_d[:], rowall[:])
            nc.sync.dma_start(z_d[:], zall[:])

    nc.compile()
    return nc


def _in_maps(h, att_feats, p_att_feats, W_h2att, b_h2att, w_alpha):
    bf = ml_dtypes.bfloat16
    att_f = np.asarray(att_feats, dtype=np.float32)
    SBF = NBF * 128
    # attbf packed [b, q, c, d]
    att_bf = np.ascontiguousarray(
        att_f[:, :SBF, :].reshape(-1, NBF, 128, D).transpose(0, 2, 1, 3)
    ).astype(bf)
    att_q8 = att_f[:, SBF:, :]
    amax = np.abs(att_q8).max(axis=2, keepdims=True)
    scale = (amax / 127.0).astype(np.float32)
    att_q = np.clip(np.round(att_q8 / scale), -127, 127).astype(np.int8)
    # att8 packed [b, q, pair, u, d]
    att_q = np.ascontiguousarray(
        att_q.reshape(-1, NQ // 2, 2, 128, D).transpose(0, 3, 1, 2, 4)
    )
    scale_bsc = scale[:, :, 0].reshape(-1, B_LOC, NQ, 128)   # [cores, b, cq, q]
    # pT2 [b, q, hc, s]: pT2[b, q, hc, s] = p[b, s, hc*128+q]
    pT = np.swapaxes(p_att_feats, 1, 2).reshape(-1, NHC, 128, S)  # [B, hc, q, s]
    pT2 = np.ascontiguousarray(np.swapaxes(pT, 1, 2)).astype(bf)  # [B, q, hc, s]
    # WT2 [q, kg, h] = W.T[kg*128+q, h]
    WT = np.asarray(W_h2att).T.reshape(KG, 128, HID)         # [kg, q, h]
    WT2 = np.ascontiguousarray(np.swapaxes(WT, 0, 1)).astype(bf)  # [q, kg, h]
    wa = np.ascontiguousarray(
        w_alpha.astype(np.float32).reshape(NHC, 128).T
    ).astype(bf)
    bh = np.ascontiguousarray(
        b_h2att.astype(np.float32).reshape(NHC, 128).T
    )
    maps = []
    for c in range(8):
        sl = slice(c * B_LOC, (c + 1) * B_LOC)
        # hT2 [q, kg, b] = h[sl].T[kg*128+q, b]
        hT = np.asarray(h)[sl].T.reshape(KG, 128, B_LOC)
        hT2 = np.ascontiguousarray(np.swapaxes(hT, 0, 1)).astype(bf)
        maps.append(
            {
                "pT2": np.ascontiguousarray(pT2[sl]),
                "attbf": np.ascontiguousarray(att_bf[sl]),
                "att8": np.ascontiguousarray(att_q[sl]),
                "scl": np.ascontiguousarray(scale_bsc[c].transpose(2, 0, 1)),
                "hT2": hT2,
                "WT2": WT2,
                "wa": wa,
                "bh": bh,
            }
        )
    return maps


def kernel(h, att_feats, p_att_feats, W_h2att, b_h2att, w_alpha, b_alpha):
    global _NC_CACHE
    h = np.asarray(h)
    att_feats = np.asarray(att_feats)
    p_att_feats = np.asarray(p_att_feats)
    W_h2att = np.asarray(W_h2att)
    b_h2att = np.asarray(b_h2att)
    w_alpha = np.asarray(w_alpha)
    if _NC_CACHE is None:
        _NC_CACHE = build_kernel()
    nc = _NC_CACHE
    maps = _in_maps(h, att_feats, p_att_feats, W_h2att, b_h2att, w_alpha)
    res = run_bass_kernel_spmd(nc, maps, core_ids=list(range(8)))
    outs = []
    for c in range(8):
        row = res.results[c]["out"]                     # [8, 2048] unnormalized
        z = res.results[c]["zall"].sum(axis=0)          # [8]
        outs.append(row / z[:, None])
    return np.concatenate(outs, axis=0).astype(np.float32)


# revision 18
# speedup vs baseline: 1.0011x; 1.0011x over previous
"""Trainium2 Bass kernel for the attention module:

    att_h  = h @ W_h2att.T + b_h2att             # [B, 512]
    dot    = tanh(p_att_feats + att_h[:, None])  # [B, 1024, 512]
    scores = dot @ w_alpha + b_alpha             # [B, 1024]
    weight = softmax(scores, axis=1)
    out    = einsum('bs,bsd->bd', weight, att_feats)  # [B, 2048]

Sharding: data-parallel over batch B=64 across 8 NeuronCores (8 per core).
Params tiny + replicated. b_alpha is a softmax shift -> dropped.

v10 design (~32MB/core HBM read; DMA floor ~95us):
  - hybrid att precision: s-chunks c=0,1 stay bf16 (consumed directly);
    c=2..7 host-quantized int8 + per-(b,s)-row absmax/127 scale.
    Upconverts: ACT does c2,c3 (early-landing), DVE does c4..c7.
    Pool only issues pT DMAs (its software tensor_copy is ~3x the
    cost-model estimate and poisons the tile schedule).
  - every DMA has >=2KB contiguous per-partition lines (WT/hT/pT are
    host-arranged partition-major; fragmented APs run ~4x slower).
  - int8 pools split by reader engine (a8A/a8D) so one engine's lag
    can't stall the other's ring recycle.
  - int8 chunks DMA first / bf16 last per batch; the weighted matmuls
    consume in the same order, hiding convert latency.
  - weighted sum accumulates all batches into persistent [8, 512] PSUM
    bank tiles via zero-masked weight columns (lhsT [128, 8], only
    column b nonzero); single tail copy + one output DMA.
  - ACT computes tanh(pT + att_h[b,h]) with att_h as per-partition bias;
    scores via PE matvec (s = c*128 + q); exp 2 iterations ahead of use.
  - DMA queues: att on SP ring, wt on ACT ring, pT/consts on gpsimd.
"""

import numpy as np
import ml_dtypes

import concourse.bass as bass
import concourse.tile as tile
from concourse import bacc, mybir
from concourse.bass import ts
from concourse.bass_utils import run_bass_kernel_spmd

F32 = mybir.dt.float32
BF16 = mybir.dt.bfloat16
I8 = mybir.dt.int8

B_LOC = 8       # batches per core
S = 1024        # attended positions
NC_ = 8         # s-chunks (s = c*128 + q)
NBF = 2         # bf16-direct s-chunks per batch (c = 0..NBF-1)
NQ = NC_ - NBF  # int8 s-chunks per batch (c = NBF..7)
HID = 512
NHC = 4         # h-chunks
D = 2048
DT = D // 512   # output column slices
K = 2048        # rnn_size contraction
KG = K // 128   # 16 k-groups

_NC_CACHE = None


def build_kernel(abf_bufs=4, a8a_bufs=4, a8d_bufs=8, aba_bufs=4, abd_bufs=11,
                 pt_bufs=2, th_bufs=2):
    nc = bacc.Bacc("TRN2", target_bir_lowering=False, debug=False, num_devices=8)

    # pT2 host layout: [b, q, hc, s] (partition-major, 8KB lines)
    p_d = nc.dram_tensor("pT2", [B_LOC, 128, NHC, S], BF16, kind="ExternalInput")
    # packed layouts: per-partition lines are contiguous (8KB / 4KB)
    abf_d = nc.dram_tensor("attbf", [B_LOC, 128, NBF, D], BF16, kind="ExternalInput")
    a8_d = nc.dram_tensor("att8", [B_LOC, 128, NQ // 2, 2, D], I8, kind="ExternalInput")
    scl_d = nc.dram_tensor("scl", [128, B_LOC, NQ], F32, kind="ExternalInput")
    # hT2 host layout: [q, kg, b]; WT2 host layout: [q, kg, h]
    hT_d = nc.dram_tensor("hT2", [128, KG, B_LOC], BF16, kind="ExternalInput")
    WT_d = nc.dram_tensor("WT2", [128, KG, HID], BF16, kind="ExternalInput")
    wa_d = nc.dram_tensor("wa", [128, NHC], BF16, kind="ExternalInput")
    bh_d = nc.dram_tensor("bh", [128, NHC], F32, kind="ExternalInput")
    out_d = nc.dram_tensor("out", [B_LOC, D], F32, kind="ExternalOutput")
    z_d = nc.dram_tensor("zall", [128, B_LOC], F32, kind="ExternalOutput")

    with tile.TileContext(nc) as tc:
        with (
            tc.tile_pool(name="consts", bufs=1) as consts,
            tc.tile_pool(name="singles", bufs=1) as singles,
            tc.tile_pool(name="ptp", bufs=pt_bufs) as ptpool,
            tc.tile_pool(name="thp", bufs=th_bufs) as thpool,
            tc.tile_pool(name="wgtp", bufs=3) as wgtpool,
            tc.tile_pool(name="wgmp", bufs=3) as wgmpool,
            tc.tile_pool(name="abfp", bufs=abf_bufs) as abfpool,
            tc.tile_pool(name="a8ap", bufs=a8a_bufs) as a8apool,
            tc.tile_pool(name="a8dp", bufs=a8d_bufs) as a8dpool,
            tc.tile_pool(name="abap", bufs=aba_bufs) as abapool,
            tc.tile_pool(name="abdp", bufs=abd_bufs) as abdpool,
            tc.tile_pool(name="ps_ah", bufs=1, space=bass.MemorySpace.PSUM) as ps_ah,
            tc.tile_pool(name="ps_sc", bufs=2, space=bass.MemorySpace.PSUM) as ps_sc,
            tc.tile_pool(name="ps_acc", bufs=1, space=bass.MemorySpace.PSUM) as ps_acc,
        ):
            abf_tiles = {}
            a8_tiles = {}

            def emit_att_dma(b):
                # int8 pairs on the SP ring (4KB lines); the bf16 pair c0-1
                # goes on the ACT ring (8KB lines) which also carried wt
                ta = a8apool.tile([128, 2, D], I8, name=f"a8a{b}", tag="a8a")
                nc.sync.dma_start(ta[:], a8_d[b][:, 0, :, :])
                t1 = a8dpool.tile([128, 2, D], I8, name=f"a8d{b}_0", tag="a8d")
                nc.sync.dma_start(t1[:], a8_d[b][:, 1, :, :])
                t2 = a8dpool.tile([128, 2, D], I8, name=f"a8d{b}_1", tag="a8d")
                nc.sync.dma_start(t2[:], a8_d[b][:, 2, :, :])
                a8_tiles[b] = [ta, t1, t2]
                at = abfpool.tile([128, NBF, D], BF16, name=f"abf{b}", tag="abf")
                nc.scalar.dma_start(at[:], abf_d[b])
                abf_tiles[b] = at

            ab_tiles = {}

            def emit_convert(b, engines="AD"):
                tiles = ab_tiles.setdefault(b, {})
                for c in range(NBF, NC_):
                    i = c - NBF
                    eng = "A" if c == 2 else "D"
                    if eng not in engines:
                        continue
                    src = a8_tiles[b][i // 2][:, i % 2, :]
                    if eng == "A":
                        dst = abapool.tile([128, D], BF16, name=f"aba{b}_{c}", tag="aba")
                        nc.scalar.copy(dst[:], src)
                    else:
                        dst = abdpool.tile([128, D], BF16, name=f"abd{b}_{c}", tag="abd")
                        nc.vector.tensor_copy(dst[:], src)
                    tiles[c] = dst

            pt_tiles = {}

            def emit_pt_dma(b):
                pt = ptpool.tile([128, NHC, S], BF16, name=f"pt{b}", tag="pt")
                nc.gpsimd.dma_start(pt[:], p_d[b])
                pt_tiles[b] = pt

            # ---- consts: tiny ones on gpsimd ring, wt on ACT ring;
            # the att stream starts immediately on SP
            ht = consts.tile([128, KG, B_LOC], BF16)
            nc.gpsimd.dma_start(ht[:], hT_d[:])
            wa = consts.tile([128, NHC], BF16)
            nc.gpsimd.dma_start(wa[:], wa_d[:])
            bh = consts.tile([128, NHC], F32)
            nc.gpsimd.dma_start(bh[:], bh_d[:])
            scl = consts.tile([128, B_LOC, NQ], F32)
            nc.gpsimd.dma_start(scl[:], scl_d[:])

            wt = consts.tile([128, KG, HID], BF16)
            nc.scalar.dma_start(wt[:], WT_d[:])

            emit_att_dma(0)
            emit_pt_dma(0)
            emit_att_dma(1)
            emit_pt_dma(1)

            # ---- att_hT[h, b] = sum_k WT[k, h] * hT[k, b]  ([128, 4hc, 8b])
            ahT = ps_ah.tile([128, NHC * B_LOC], F32)
            for hc in range(NHC):
                for kg in range(KG):
                    nc.tensor.matmul(
                        ahT[:, ts(hc, B_LOC)],
                        wt[:, kg, ts(hc, 128)],
                        ht[:, kg, :],
                        start=(kg == 0),
                        stop=(kg == KG - 1),
                    )
            biasT = singles.tile([128, NHC, B_LOC], F32)
            for hc in range(NHC):
                nc.vector.tensor_add(
                    biasT[:, hc, :],
                    ahT[:, ts(hc, B_LOC)],
                    bh[:, hc : hc + 1].broadcast_to((128, B_LOC)),
                )

            # per-partition exp partial sums, one column per batch
            zall = singles.tile([128, B_LOC], F32)

            # persistent weighted-sum accumulator: [8 batches, 4 dchunks, 512]
            acc = ps_acc.tile([B_LOC, DT, 512], F32)

            th_tiles = {}
            sc_tiles = {}
            wgt_tiles = {}
            wgm_tiles = {}

            def emit_tanh(b):
                th = thpool.tile([128, NHC, S], BF16, name=f"th{b}", tag="th")
                for hc in range(NHC):
                    nc.scalar.activation(
                        th[:, hc, :], pt_tiles[b][:, hc, :],
                        mybir.ActivationFunctionType.Tanh,
                        bias=biasT[:, hc, b : b + 1], scale=1.0,
                    )
                th_tiles[b] = th

            def emit_scores(b):
                sc = ps_sc.tile([128, NC_], F32, name=f"sc{b}", tag="sc")
                for c in range(NC_):
                    for hc in range(NHC):
                        nc.tensor.matmul(
                            sc[:, c : c + 1],
                            th_tiles[b][:, hc, ts(c, 128)],
                            wa[:, hc : hc + 1],
                            start=(hc == 0),
                            stop=(hc == NHC - 1),
                        )
                sc_tiles[b] = sc

            def emit_exp(b):
                wgt = wgtpool.tile([128, NC_], BF16, name=f"wgt{b}", tag="wgt")
                nc.scalar.activation(
                    wgt[:], sc_tiles[b][:], mybir.ActivationFunctionType.Exp,
                    accum_out=zall[:, b : b + 1],
                )
                wgt_tiles[b] = wgt

            def emit_wgm(b):
                # masked lhsT: [128, c, 8] zero except column b; int8 chunks
                # get the dequant row scale folded in here
                wgm = wgmpool.tile(
                    [128, NC_, B_LOC], BF16, name=f"wgm{b}", tag="wgm"
                )
                nc.vector.memset(wgm[:], 0.0)
                nc.vector.tensor_copy(
                    wgm[:, 0:NBF, b], wgt_tiles[b][:, 0:NBF]
                )
                nc.vector.tensor_mul(
                    wgm[:, NBF:NC_, b], wgt_tiles[b][:, NBF:NC_], scl[:, b, :]
                )
                wgm_tiles[b] = wgm

            def emit_weighted(b):
                for c in [2, 3, 4, 5, 6, 7, 0, 1]:
                    if c < NBF:
                        at = abf_tiles[b][:, c, :]
                    else:
                        at = ab_tiles[b][c][:]
                    for d in range(DT):
                        nc.tensor.matmul(
                            acc[:, d, :],
                            wgm_tiles[b][:, c, :],
                            at[:, ts(d, 512)],
                            start=(b == 0 and c == 2),
                            stop=(b == B_LOC - 1 and c == 1),
                            skip_group_check=True,
                        )

            # ---- prologue
            emit_convert(0, "D")
            emit_tanh(0)
            emit_scores(0)
            emit_exp(0)
            emit_wgm(0)
            emit_convert(0, "A")
            emit_tanh(1)
            emit_scores(1)
            emit_exp(1)
            emit_wgm(1)
            emit_convert(1, "D")

            for b in range(B_LOC):
                if b + 2 < B_LOC:
                    emit_att_dma(b + 2)
                    emit_pt_dma(b + 2)
                emit_weighted(b)
                if b + 2 < B_LOC:
                    emit_tanh(b + 2)
                if b + 1 < B_LOC:
                    emit_convert(b + 1, "AD" if b > 0 else "A")
                if b + 2 < B_LOC:
                    emit_scores(b + 2)
                    emit_exp(b + 2)
                    emit_wgm(b + 2)

            # tail: drain the persistent accumulator
            rowall = ptpool.tile([B_LOC, D], F32)
            for d in range(DT):
                nc.vector.tensor_copy(rowall[:, ts(d, 512)], acc[:, d, :])
            nc.sync.dma_start(out_d[:], rowall[:])
            nc.sync.dma_start(z_d[:], zall[:])

    nc.compile()
    return nc


def _in_maps(h, att_feats, p_att_feats, W_h2att, b_h2att, w_alpha):
    bf = ml_dtypes.bfloat16
    att_f = np.asarray(att_feats, dtype=np.float32)
    SBF = NBF * 128
    # attbf packed [b, q, c, d]
    att_bf = np.ascontiguousarray(
        att_f[:, :SBF, :].reshape(-1, NBF, 128, D).transpose(0, 2, 1, 3)
    ).astype(bf)
    att_q8 = att_f[:, SBF:, :]
    amax = np.abs(att_q8).max(axis=2, keepdims=True)
    scale = (amax / 127.0).astype(np.float32)
    att_q = np.clip(np.round(att_q8 / scale), -127, 127).astype(np.int8)
    # att8 packed [b, q, pair, u, d]
    att_q = np.ascontiguousarray(
        att_q.reshape(-1, NQ // 2, 2, 128, D).transpose(0, 3, 1, 2, 4)
    )
    scale_bsc = scale[:, :, 0].reshape(-1, B_LOC, NQ, 128)   # [cores, b, cq, q]
    # pT2 [b, q, hc, s]: pT2[b, q, hc, s] = p[b, s, hc*128+q]
    pT = np.swapaxes(p_att_feats, 1, 2).reshape(-1, NHC, 128, S)  # [B, hc, q, s]
    pT2 = np.ascontiguousarray(np.swapaxes(pT, 1, 2)).astype(bf)  # [B, q, hc, s]
    # WT2 [q, kg, h] = W.T[kg*128+q, h]
    WT = np.asarray(W_h2att).T.reshape(KG, 128, HID)         # [kg, q, h]
    WT2 = np.ascontiguousarray(np.swapaxes(WT, 0, 1)).astype(bf)  # [q, kg, h]
    wa = np.ascontiguousarray(
        w_alpha.astype(np.float32).reshape(NHC, 128).T
    ).astype(bf)
    bh = np.ascontiguousarray(
        b_h2att.astype(np.float32).reshape(NHC, 128).T
    )
    maps = []
    for c in range(8):
        sl = slice(c * B_LOC, (c + 1) * B_LOC)
        # hT2 [q, kg, b] = h[sl].T[kg*128+q, b]
        hT = np.asarray(h)[sl].T.reshape(KG, 128, B_LOC)
        hT2 = np.ascontiguousarray(np.swapaxes(hT, 0, 1)).astype(bf)
        maps.append(
            {
                "pT2": np.ascontiguousarray(pT2[sl]),
                "attbf": np.ascontiguousarray(att_bf[sl]),
                "att8": np.ascontiguousarray(att_q[sl]),
                "scl": np.ascontiguousarray(scale_bsc[c].transpose(2, 0, 1)),
                "hT2": hT2,
                "WT2": WT2,
                "wa": wa,
                "bh": bh,
            }
        )
    return maps


def kernel(h, att_feats, p_att_feats, W_h2att, b_h2att, w_alpha, b_alpha):
    global _NC_CACHE
    h = np.asarray(h)
    att_feats = np.asarray(att_feats)
    p_att_feats = np.asarray(p_att_feats)
    W_h2att = np.asarray(W_h2att)
    b_h2att = np.asarray(b_h2att)
    w_alpha = np.asarray(w_alpha)
    if _NC_CACHE is None:
        _NC_CACHE = build_kernel()
    nc = _NC_CACHE
    maps = _in_maps(h, att_feats, p_att_feats, W_h2att, b_h2att, w_alpha)
    res = run_bass_kernel_spmd(nc, maps, core_ids=list(range(8)))
    outs = []
    for c in range(8):
        row = res.results[c]["out"]                     # [8, 2048] unnormalized
        z = res.results[c]["zall"].sum(axis=0)          # [8]
        outs.append(row / z[:, None])
    return np.concatenate(outs, axis=0).astype(np.float32)


# revision 19
# speedup vs baseline: 1.1024x; 1.1011x over previous
"""Trainium2 Bass kernel for the attention module:

    att_h  = h @ W_h2att.T + b_h2att             # [B, 512]
    dot    = tanh(p_att_feats + att_h[:, None])  # [B, 1024, 512]
    scores = dot @ w_alpha + b_alpha             # [B, 1024]
    weight = softmax(scores, axis=1)
    out    = einsum('bs,bsd->bd', weight, att_feats)  # [B, 2048]

Sharding: data-parallel over batch B=64 across 8 NeuronCores (8 per core).
Params tiny + replicated. b_alpha is a softmax shift -> dropped.

v10 design (~32MB/core HBM read; DMA floor ~95us):
  - hybrid att precision: s-chunks c=0,1 stay bf16 (consumed directly);
    c=2..7 host-quantized int8 + per-(b,s)-row absmax/127 scale.
    Upconverts: ACT does c2,c3 (early-landing), DVE does c4..c7.
    Pool only issues pT DMAs (its software tensor_copy is ~3x the
    cost-model estimate and poisons the tile schedule).
  - every DMA has >=2KB contiguous per-partition lines (WT/hT/pT are
    host-arranged partition-major; fragmented APs run ~4x slower).
  - int8 pools split by reader engine (a8A/a8D) so one engine's lag
    can't stall the other's ring recycle.
  - int8 chunks DMA first / bf16 last per batch; the weighted matmuls
    consume in the same order, hiding convert latency.
  - weighted sum accumulates all batches into persistent [8, 512] PSUM
    bank tiles via zero-masked weight columns (lhsT [128, 8], only
    column b nonzero); single tail copy + one output DMA.
  - ACT computes tanh(pT + att_h[b,h]) with att_h as per-partition bias;
    scores via PE matvec (s = c*128 + q); exp 2 iterations ahead of use.
  - DMA queues: att on SP ring, wt on ACT ring, pT/consts on gpsimd.
"""

import numpy as np
import ml_dtypes

import concourse.bass as bass
import concourse.tile as tile
from concourse import bacc, mybir
from concourse.bass import ts
from concourse.bass_utils import run_bass_kernel_spmd

F32 = mybir.dt.float32
BF16 = mybir.dt.bfloat16
I8 = mybir.dt.int8
F8 = mybir.dt.float8e4

B_LOC = 8       # batches per core
S = 1024        # attended positions
NC_ = 8         # s-chunks (s = c*128 + q)
NBF = 2         # bf16-direct s-chunks per batch (c = 0..NBF-1)
NQ = NC_ - NBF  # int8 s-chunks per batch (c = NBF..7)
HID = 512
NHC = 4         # h-chunks
D = 2048
DT = D // 512   # output column slices
K = 2048        # rnn_size contraction
KG = K // 128   # 16 k-groups

_NC_CACHE = None


def build_kernel(abf_bufs=4, a8a_bufs=5, a8d_bufs=9, aba_bufs=5, abd_bufs=12,
                 pt_bufs=2, th_bufs=2):
    nc = bacc.Bacc("TRN2", target_bir_lowering=False, debug=False, num_devices=8)

    # pT2 host layout: [b, q, hc, s] (partition-major, 8KB lines)
    p_d = nc.dram_tensor("pT2", [B_LOC, 128, NHC, S], BF16, kind="ExternalInput")
    # packed layouts: per-partition lines are contiguous (8KB / 4KB)
    abf_d = nc.dram_tensor("attbf", [B_LOC, 128, NBF, D], F8, kind="ExternalInput")
    a8_d = nc.dram_tensor("att8", [B_LOC, 128, NQ // 2, 2, D], I8, kind="ExternalInput")
    scl_d = nc.dram_tensor("scl", [128, B_LOC, NQ], F32, kind="ExternalInput")
    # hT2 host layout: [q, kg, b]; WT2 host layout: [q, kg, h]
    hT_d = nc.dram_tensor("hT2", [128, KG, B_LOC], BF16, kind="ExternalInput")
    WT_d = nc.dram_tensor("WT2", [128, KG, HID], BF16, kind="ExternalInput")
    wa_d = nc.dram_tensor("wa", [128, NHC], BF16, kind="ExternalInput")
    bh_d = nc.dram_tensor("bh", [128, NHC], F32, kind="ExternalInput")
    out_d = nc.dram_tensor("out", [B_LOC, D], F32, kind="ExternalOutput")
    z_d = nc.dram_tensor("zall", [128, B_LOC], F32, kind="ExternalOutput")

    with tile.TileContext(nc) as tc:
        with (
            tc.tile_pool(name="consts", bufs=1) as consts,
            tc.tile_pool(name="singles", bufs=1) as singles,
            tc.tile_pool(name="ptp", bufs=pt_bufs) as ptpool,
            tc.tile_pool(name="thp", bufs=th_bufs) as thpool,
            tc.tile_pool(name="wgtp", bufs=3) as wgtpool,
            tc.tile_pool(name="wgmp", bufs=3) as wgmpool,
            tc.tile_pool(name="abfp", bufs=abf_bufs) as abfpool,
            tc.tile_pool(name="a8ap", bufs=a8a_bufs) as a8apool,
            tc.tile_pool(name="a8dp", bufs=a8d_bufs) as a8dpool,
            tc.tile_pool(name="abap", bufs=aba_bufs) as abapool,
            tc.tile_pool(name="abdp", bufs=abd_bufs) as abdpool,
            tc.tile_pool(name="ps_ah", bufs=1, space=bass.MemorySpace.PSUM) as ps_ah,
            tc.tile_pool(name="ps_sc", bufs=2, space=bass.MemorySpace.PSUM) as ps_sc,
            tc.tile_pool(name="ps_acc", bufs=1, space=bass.MemorySpace.PSUM) as ps_acc,
        ):
            abf_tiles = {}
            a8_tiles = {}

            def emit_att_dma(b):
                # int8 pairs on the SP ring (4KB lines); the bf16 pair c0-1
                # goes on the ACT ring (8KB lines) which also carried wt
                ta = a8apool.tile([128, 2, D], I8, name=f"a8a{b}", tag="a8a")
                nc.sync.dma_start(ta[:], a8_d[b][:, 0, :, :])
                t1 = a8dpool.tile([128, 2, D], I8, name=f"a8d{b}_0", tag="a8d")
                nc.sync.dma_start(t1[:], a8_d[b][:, 1, :, :])
                t2 = a8dpool.tile([128, 2, D], I8, name=f"a8d{b}_1", tag="a8d")
                nc.sync.dma_start(t2[:], a8_d[b][:, 2, :, :])
                a8_tiles[b] = [ta, t1, t2]
                at = abfpool.tile([128, NBF, D], F8, name=f"abf{b}", tag="abf")
                nc.scalar.dma_start(at[:], abf_d[b])
                abf_tiles[b] = at

            ab_tiles = {}

            def emit_convert(b, engines="AD"):
                tiles = ab_tiles.setdefault(b, {})
                for c in range(NBF, NC_):
                    i = c - NBF
                    eng = "A" if c == 2 else "D"
                    if eng not in engines:
                        continue
                    src = a8_tiles[b][i // 2][:, i % 2, :]
                    if eng == "A":
                        dst = abapool.tile([128, D], BF16, name=f"aba{b}_{c}", tag="aba")
                        nc.scalar.copy(dst[:], src)
                    else:
                        dst = abdpool.tile([128, D], BF16, name=f"abd{b}_{c}", tag="abd")
                        nc.vector.tensor_copy(dst[:], src)
                    tiles[c] = dst

            pt_tiles = {}

            def emit_pt_dma(b):
                pt = ptpool.tile([128, NHC, S], BF16, name=f"pt{b}", tag="pt")
                nc.gpsimd.dma_start(pt[:], p_d[b])
                pt_tiles[b] = pt

            # ---- consts: tiny ones on gpsimd ring, wt on ACT ring;
            # the att stream starts immediately on SP
            ht = consts.tile([128, KG, B_LOC], BF16)
            nc.gpsimd.dma_start(ht[:], hT_d[:])
            wa = consts.tile([128, NHC], BF16)
            nc.gpsimd.dma_start(wa[:], wa_d[:])
            bh = consts.tile([128, NHC], F32)
            nc.gpsimd.dma_start(bh[:], bh_d[:])
            scl = consts.tile([128, B_LOC, NQ], F32)
            nc.gpsimd.dma_start(scl[:], scl_d[:])

            wt = consts.tile([128, KG, HID], BF16)
            nc.scalar.dma_start(wt[:], WT_d[:])

            emit_att_dma(0)
            emit_pt_dma(0)
            emit_att_dma(1)
            emit_pt_dma(1)

            # ---- att_hT[h, b] = sum_k WT[k, h] * hT[k, b]  ([128, 4hc, 8b])
            ahT = ps_ah.tile([128, NHC * B_LOC], F32)
            for hc in range(NHC):
                for kg in range(KG):
                    nc.tensor.matmul(
                        ahT[:, ts(hc, B_LOC)],
                        wt[:, kg, ts(hc, 128)],
                        ht[:, kg, :],
                        start=(kg == 0),
                        stop=(kg == KG - 1),
                    )
            biasT = singles.tile([128, NHC, B_LOC], F32)
            for hc in range(NHC):
                nc.vector.tensor_add(
                    biasT[:, hc, :],
                    ahT[:, ts(hc, B_LOC)],
                    bh[:, hc : hc + 1].broadcast_to((128, B_LOC)),
                )

            # per-partition exp partial sums, one column per batch
            zall = singles.tile([128, B_LOC], F32)

            # persistent weighted-sum accumulator: [8 batches, 4 dchunks, 512]
            acc = ps_acc.tile([B_LOC, DT, 512], F32)

            th_tiles = {}
            sc_tiles = {}
            wgt_tiles = {}
            wgm_tiles = {}

            def emit_tanh(b):
                th = thpool.tile([128, NHC, S], BF16, name=f"th{b}", tag="th")
                for hc in range(NHC):
                    nc.scalar.activation(
                        th[:, hc, :], pt_tiles[b][:, hc, :],
                        mybir.ActivationFunctionType.Tanh,
                        bias=biasT[:, hc, b : b + 1], scale=1.0,
                    )
                th_tiles[b] = th

            def emit_scores(b):
                sc = ps_sc.tile([128, NC_], F32, name=f"sc{b}", tag="sc")
                for c in range(NC_):
                    for hc in range(NHC):
                        nc.tensor.matmul(
                            sc[:, c : c + 1],
                            th_tiles[b][:, hc, ts(c, 128)],
                            wa[:, hc : hc + 1],
                            start=(hc == 0),
                            stop=(hc == NHC - 1),
                        )
                sc_tiles[b] = sc

            def emit_exp(b):
                wgt = wgtpool.tile([128, NC_], BF16, name=f"wgt{b}", tag="wgt")
                nc.scalar.activation(
                    wgt[:], sc_tiles[b][:], mybir.ActivationFunctionType.Exp,
                    accum_out=zall[:, b : b + 1],
                )
                wgt_tiles[b] = wgt

            def emit_wgm(b):
                # masked lhsT: [128, c, 8] zero except column b; int8 chunks
                # get the dequant row scale folded in here
                wgm = wgmpool.tile(
                    [128, NC_, B_LOC], BF16, name=f"wgm{b}", tag="wgm"
                )
                nc.vector.memset(wgm[:], 0.0)
                nc.vector.tensor_copy(
                    wgm[:, 0:NBF, b], wgt_tiles[b][:, 0:NBF]
                )
                nc.vector.tensor_mul(
                    wgm[:, NBF:NC_, b], wgt_tiles[b][:, NBF:NC_], scl[:, b, :]
                )
                wgm_tiles[b] = wgm

            def emit_weighted(b):
                for c in [2, 3, 4, 5, 6, 7, 0, 1]:
                    if c < NBF:
                        at = abf_tiles[b][:, c, :]
                    else:
                        at = ab_tiles[b][c][:]
                    for d in range(DT):
                        nc.tensor.matmul(
                            acc[:, d, :],
                            wgm_tiles[b][:, c, :],
                            at[:, ts(d, 512)],
                            start=(b == 0 and c == 2),
                            stop=(b == B_LOC - 1 and c == 1),
                            skip_group_check=True,
                        )

            # ---- prologue
            emit_convert(0, "D")
            emit_tanh(0)
            emit_scores(0)
            emit_exp(0)
            emit_wgm(0)
            emit_convert(0, "A")
            emit_tanh(1)
            emit_scores(1)
            emit_exp(1)
            emit_wgm(1)
            emit_convert(1, "D")

            for b in range(B_LOC):
                if b + 2 < B_LOC:
                    emit_att_dma(b + 2)
                    emit_pt_dma(b + 2)
                emit_weighted(b)
                if b + 2 < B_LOC:
                    emit_tanh(b + 2)
                if b + 1 < B_LOC:
                    emit_convert(b + 1, "AD" if b > 0 else "A")
                if b + 2 < B_LOC:
                    emit_scores(b + 2)
                    emit_exp(b + 2)
                    emit_wgm(b + 2)

            # tail: drain the persistent accumulator
            rowall = ptpool.tile([B_LOC, D], F32)
            for d in range(DT):
                nc.vector.tensor_copy(rowall[:, ts(d, 512)], acc[:, d, :])
            nc.sync.dma_start(out_d[:], rowall[:])
            nc.sync.dma_start(z_d[:], zall[:])

    nc.compile()
    return nc


def _in_maps(h, att_feats, p_att_feats, W_h2att, b_h2att, w_alpha):
    bf = ml_dtypes.bfloat16
    att_f = np.asarray(att_feats, dtype=np.float32)
    SBF = NBF * 128
    # attbf packed [b, q, c, d], fp8-e4m3 (consumed directly by PE)
    att_bf = np.ascontiguousarray(
        att_f[:, :SBF, :].reshape(-1, NBF, 128, D).transpose(0, 2, 1, 3)
    ).astype(ml_dtypes.float8_e4m3fn)
    att_q8 = att_f[:, SBF:, :]
    amax = np.abs(att_q8).max(axis=2, keepdims=True)
    scale = (amax / 127.0).astype(np.float32)
    att_q = np.clip(np.round(att_q8 / scale), -127, 127).astype(np.int8)
    # att8 packed [b, q, pair, u, d]
    att_q = np.ascontiguousarray(
        att_q.reshape(-1, NQ // 2, 2, 128, D).transpose(0, 3, 1, 2, 4)
    )
    scale_bsc = scale[:, :, 0].reshape(-1, B_LOC, NQ, 128)   # [cores, b, cq, q]
    # pT2 [b, q, hc, s]: pT2[b, q, hc, s] = p[b, s, hc*128+q]
    pT = np.swapaxes(p_att_feats, 1, 2).reshape(-1, NHC, 128, S)  # [B, hc, q, s]
    pT2 = np.ascontiguousarray(np.swapaxes(pT, 1, 2)).astype(bf)  # [B, q, hc, s]
    # WT2 [q, kg, h] = W.T[kg*128+q, h]
    WT = np.asarray(W_h2att).T.reshape(KG, 128, HID)         # [kg, q, h]
    WT2 = np.ascontiguousarray(np.swapaxes(WT, 0, 1)).astype(bf)  # [q, kg, h]
    wa = np.ascontiguousarray(
        w_alpha.astype(np.float32).reshape(NHC, 128).T
    ).astype(bf)
    bh = np.ascontiguousarray(
        b_h2att.astype(np.float32).reshape(NHC, 128).T
    )
    maps = []
    for c in range(8):
        sl = slice(c * B_LOC, (c + 1) * B_LOC)
        # hT2 [q, kg, b] = h[sl].T[kg*128+q, b]
        hT = np.asarray(h)[sl].T.reshape(KG, 128, B_LOC)
        hT2 = np.ascontiguousarray(np.swapaxes(hT, 0, 1)).astype(bf)
        maps.append(
            {
                "pT2": np.ascontiguousarray(pT2[sl]),
                "attbf": np.ascontiguousarray(att_bf[sl]),
                "att8": np.ascontiguousarray(att_q[sl]),
                "scl": np.ascontiguousarray(scale_bsc[c].transpose(2, 0, 1)),
                "hT2": hT2,
                "WT2": WT2,
                "wa": wa,
                "bh": bh,
            }
        )
    return maps


def kernel(h, att_feats, p_att_feats, W_h2att, b_h2att, w_alpha, b_alpha):
    global _NC_CACHE
    h = np.asarray(h)
    att_feats = np.asarray(att_feats)
    p_att_feats = np.asarray(p_att_feats)
    W_h2att = np.asarray(W_h2att)
    b_h2att = np.asarray(b_h2att)
    w_alpha = np.asarray(w_alpha)
    if _NC_CACHE is None:
        _NC_CACHE = build_kernel()
    nc = _NC_CACHE
    maps = _in_maps(h, att_feats, p_att_feats, W_h2att, b_h2att, w_alpha)
    res = run_bass_kernel_spmd(nc, maps, core_ids=list(range(8)))
    outs = []
    for c in range(8):
        row = res.results[c]["out"]                     # [8, 2048] unnormalized
        z = res.results[c]["zall"].sum(axis=0)          # [8]
        outs.append(row / z[:, None])
    return np.concatenate(outs, axis=0).astype(np.float32)
